# revision 2
# baseline (speedup 1.0000x reference)
"""Trainium2 Bass kernel for nn_Conv2d_int8_STE.

Reference:
  sx = max|x|/127 ; qx = round(x/sx)
  sw = max|w|/127 ; qw = round(w/sw)
  out = conv2d(qx, qw, pad=1) * (sx*sw) + bias
The LUT is the exact int8 product table, so the conv over integer levels
reproduces it exactly.

Device pipeline (per core, one image; data-parallel over B=8):
  - x DMAed once per row-chunk with a broadcast access pattern into 96
    partitions (3 copies for the kw taps: free, DMA cost is per-partition
    bytes).
  - one-pass quantize: p = fp16(x*inv_sx + 1536). fp16 ULP is 1 on
    [1024,2048), so the f32->fp16 cast rounds to the nearest integer
    level; the +1536 offset is removed via two constant contraction rows
    (K=98) whose weights are -(1536*sum qw - bias/s)/256 split hi/lo.
  - 3 accumulating matmuls per chunk over kh (K=98, fp16).
  - epilogue: out = psum * s_out (pure scale; bias already in psum).
  - PE warmup matmuls at t~400 pin pe_busy_start early so real matmuls
    run at full clock.
"""

import os
import sys

for _p in ("/opt/trn_rl_repo", "/root/.axon_site/_ro/trn_rl_repo"):
    if os.path.isdir(_p) and _p not in sys.path:
        sys.path.insert(0, _p)

import numpy as np

import concourse.bass as bass
import concourse.tile as tile
from concourse import bacc, mybir
from concourse.bass_utils import run_bass_kernel_spmd

F32 = mybir.dt.float32
F16 = mybir.dt.float16
MULT = mybir.AluOpType.mult
ADD = mybir.AluOpType.add
COPY = mybir.ActivationFunctionType.Copy

B, CIN, H, W = 8, 32, 32, 32
COUT, KH, KW = 32, 3, 3
PW = W + 2
PH = H + 2
PHW = PW * PH
OHW = H * W
K96 = KW * CIN          # 96 data contraction rows
KTOT = K96
MAGIC = 1536.0          # fp16 round-at-integer offset (ULP=1 in [1024,2048))

N_CORES = 8
_CACHE = {}

# chunk row ranges (x rows); chunk 0 carries the top halo rows
XR0 = [0, 9, 17, 25]
XNR = [9, 8, 8, 7]
R = 8
# process order: by input-DMA arrival (SP c0, ACT c3, Pool c2, SP2 c1)
PROC = [0, 1, 2, 3]


def _build_program(inv_sx, inv_sw, s_out):
    nc = bacc.Bacc("TRN2", target_bir_lowering=False, debug=False,
                   num_devices=N_CORES)

    x_d = nc.dram_tensor("x", [CIN, OHW], F32, kind="ExternalInput")
    wt_d = nc.dram_tensor("wt", [KTOT, KH * COUT], F16, kind="ExternalInput")
    aux_d = nc.dram_tensor("aux", [COUT, 1], F32, kind="ExternalInput")
    out_d = nc.dram_tensor("out", [COUT, OHW], F32, kind="ExternalOutput")

    with tile.TileContext(nc) as tc:
        with (
            tc.tile_pool(name="sbuf", bufs=1) as pool,
            tc.tile_pool(name="psum", bufs=1, space="PSUM") as psum,
        ):
            wq = pool.tile([KTOT, KH * COUT], F16)
            praw = pool.tile([K96, OHW], F32)
            p = pool.tile([KTOT, PHW], F16)
            p_rows = p[:].rearrange("p (r c) -> p r c", c=PW)

            dummy = pool.tile([1, 64], F16, name="dummy", tag="dummy")
            psw = psum.tile([1, 64], F32, name="psw", tag="psw")

            # ---- t~100: DMA issues ----
            # SP: in c0, const rows; ACT: wt, in c3; Pool: in c2, in c1
            def in_dma(eng, c):
                src = x_d.ap()[:, XR0[c] * W:(XR0[c] + XNR[c]) * W]
                srcb = src.unsqueeze(0).broadcast_to([KW, CIN, XNR[c] * W])
                eng.dma_start(praw[:, XR0[c] * W:(XR0[c] + XNR[c]) * W], srcb)

            aux = pool.tile([COUT, 1], F32)
            bias_adj = aux[:, 0:1]
            fill = pool.tile([1, 160], F32, name="fill", tag="fill")
            in_dma(nc.sync, 0)
            nc.scalar.dma_start(wq[:], wt_d.ap())
            in_dma(nc.gpsimd, 1)
            in_dma(nc.scalar, 3)
            in_dma(nc.sync, 2)
            nc.sync.dma_start(aux[:], aux_d.ap())
            # filler: keeps Pool busy past SP's c0 issue-end so the scheduler
            # skips the DMA-completion semaphore (engine-order suffices)
            nc.gpsimd.memset(fill[:], 0.0)

            # ---- warmup PE to pin pe_busy_start early ----
            nc.vector.memset(dummy[:], 1.0)
            for _ in range(2):
                nc.tensor.matmul(psw[:], dummy[:, 0:1], dummy[:],
                                 start=True, stop=True)

            # ---- border memsets (idle window before inputs land) ----
            # top + bottom pad rows (q=0 -> value MAGIC)
            nc.vector.memset(p[0:K96, 0:W], MAGIC)
            nc.vector.memset(p[0:K96, (PH - 1) * PW:(PH - 1) * PW + W], MAGIC)
            # g0 left pad column; g2 right pad column
            nc.vector.memset(
                p[0:CIN, PW:PW + PW * H].rearrange(
                    "p (r c) -> p r c", c=PW)[:, :, 0:1], MAGIC)
            nc.vector.memset(
                p[2 * CIN:3 * CIN, PW + 31:PW + 31 + PW * H].rearrange(
                    "p (r c) -> p r c", c=PW)[:, :, 0:1], MAGIC)

            # ---- one-pass quantize, 3 engines (g0 DVE, g1 Pool, g2 split) ----
            def qdst(c, g):
                off = (XR0[c] + 1) * PW + 1 - g
                return p[g * CIN:(g + 1) * CIN, off:off + XNR[c] * PW] \
                    .rearrange("p (r c) -> p r c", c=PW)[:, :, 0:W]

            def qsrc(c, g):
                return praw[g * CIN:(g + 1) * CIN,
                            XR0[c] * W:(XR0[c] + XNR[c]) * W] \
                    .rearrange("p (r c) -> p r c", c=W)

            for c in PROC:
                nc.vector.tensor_scalar(qdst(c, 0), qsrc(c, 0),
                                        float(inv_sx), MAGIC, MULT, ADD)
                nc.vector.tensor_scalar(qdst(c, 2), qsrc(c, 2),
                                        float(inv_sx), MAGIC, MULT, ADD)
            for c in PROC:
                if c == 2:
                    # keep Pool busy past SP's c2 issue-end (no sem wait)
                    nc.gpsimd.memset(fill[:, 0:64], 1.0)
                nc.gpsimd.tensor_scalar(qdst(c, 1), qsrc(c, 1),
                                        float(inv_sx), MAGIC, MULT, ADD)

            # ---- conv: 3 accumulating matmuls per chunk ----
            ps = {}
            for c in PROC:
                ps[c] = psum.tile([COUT, R * W], F32, name=f"ps{c}",
                                  tag=f"ps{c}")
                for kh in range(KH):
                    r0 = c * R + kh
                    rhs = p_rows[:, r0:r0 + R, 0:W]
                    nc.tensor.matmul(
                        ps[c][:], wq[:, kh * COUT:(kh + 1) * COUT], rhs,
                        start=(kh == 0), stop=(kh == KH - 1))

            # ---- epilogue (scale only) + out DMA ----
            for i, c in enumerate(PROC):
                osb = pool.tile([COUT, R * W], F32, name=f"osb{c}",
                                tag=f"osb{c}")
                nc.vector.tensor_scalar(osb[:], ps[c][:], float(s_out),
                                        bias_adj, MULT, ADD)
                eng = nc.sync if i % 2 == 0 else nc.scalar
                eng.dma_start(out_d.ap()[:, c * R * W:(c + 1) * R * W], osb[:])

    nc.compile()
    return nc


def get_program(inv_sx, inv_sw, s_out):
    key = (float(inv_sx), float(inv_sw), float(s_out))
    if key not in _CACHE:
        _CACHE[key] = _build_program(*key)
    return _CACHE[key]


def _scales(x, weight):
    sx = np.float32(np.max(np.abs(x))) / np.float32(127.0)
    sw = np.float32(np.max(np.abs(weight))) / np.float32(127.0)
    inv_sx = np.float32(1.0) / sx
    inv_sw = np.float32(1.0) / sw
    return inv_sx, inv_sw, sx * sw


def make_in_maps(x, weight, bias, lut):
    x = np.asarray(x, dtype=np.float32)
    weight = np.asarray(weight, dtype=np.float32)
    bias = np.asarray(bias, dtype=np.float32)

    _, inv_sw, s_out = _scales(x, weight)
    qw = np.round(weight * inv_sw)                       # int levels, exact
    wt = np.ascontiguousarray(
        qw.transpose(3, 1, 2, 0).reshape(K96, KH * COUT)).astype(np.float16)

    # psum = conv(q, qw) + 1536*sum(qw)[cout]; fold correction into bias
    adj = (bias.astype(np.float64)
           - np.float64(MAGIC) * qw.sum(axis=(1, 2, 3)).astype(np.float64)
           * np.float64(s_out)).astype(np.float32)
    aux = np.ascontiguousarray(adj.reshape(COUT, 1))

    return [
        {"x": np.ascontiguousarray(x[b].reshape(CIN, OHW)), "wt": wt,
         "aux": aux}
        for b in range(B)
    ]


def kernel(x, weight, bias, lut, **run_kwargs):
    x = np.asarray(x, dtype=np.float32)
    weight = np.asarray(weight, dtype=np.float32)
    nc = get_program(*_scales(x, weight))
    in_maps = make_in_maps(x, weight, bias, lut)
    res = run_bass_kernel_spmd(nc, in_maps, core_ids=list(range(N_CORES)),
                               **run_kwargs)
    out = np.stack([res.results[b]["out"].reshape(COUT, H, W)
                    for b in range(B)]).astype(np.float32)
    _CACHE["last_results"] = res
    return out
